# revision 34
# baseline (speedup 1.0000x reference)
"""Trainium2 Bass kernel for nn_Block1 (dense_cnn edge-filter bank).

kernel(pan) -> [2, 6, 2048, 2048] f32: concat([diff_y, diff_x, roberts,
prewitt, sobel, laplacian], axis=1) with a global-max normalization of the
Gaussian-filtered image (see the reference nn.Module).

Distribution: data parallel over 8 NeuronCores — core c owns the
[1024 x 1024] quadrant (b, rh, ch) = (c//4, (c%4)//2, c%2) of the batch,
loaded as a reflect-padded [1028, 1028] f32 slab (direct f32r load). The
1024-col bands waste only 9x124-1024 = 92 rows of PE work per core vs the
wide-band layout's 5x124-512 = 108 per 512 rows (9% vs 21%).

All convs run as banded [128,128] matmuls on PE (21 taps per 124-row band);
abs/round/clip/scale epilogues are fused single ops spread across
ACT/DVE/Pool (Pool cannot read PSUM; abs_max tensor_scalar and int
tensor_tensor dtype-mixes are not in the ISA — hence ACT Abs + custom DVE
select-abs). All six channels store as u8 (diff_y/diff_x affine-quantized,
dequantized on the host; edge channels carry norm/255 on the host). The
per-batch global max uses an AllGather (15us vs AllReduce 28us), shadowed
by dy/sobel work, with sobel |conv| staged to f16 so PE keeps running.
"""
import sys

sys.path.insert(0, "/opt/trn_rl_repo")

import numpy as np
import ml_dtypes

import concourse.bacc as bacc
import concourse.mybir as mybir
import concourse.bass_isa as bass_isa
from concourse.tile import TileContext
from concourse.dve_ops import (DveOp, DveOpSpec, OPS, CUSTOM_DVE_SPECS,
                               _SUB_OPCODE_FOR_NAME, _CUSTOM_DVE_ROW_BASE)
from concourse.dve_spec import Spec, Src0, Src1, C0, C1, C2, One, select, lower

f32 = mybir.dt.float32
f32r = mybir.dt.float32r
bf16 = mybir.dt.bfloat16
f16 = mybir.dt.float16
i32 = mybir.dt.int32
u8 = mybir.dt.uint8

P = 128
W = 1024            # band width (output cols per core-slab row)
WP = 1028           # input slab width (2-col halo each side)
LPW = 1026          # lp/spf width (1-col halo each side)
ROWS = 1028         # input slab rows
NB = 9              # row bands per core
TSTART = [124 * t for t in range(8)] + [900]
STORE = [(0, 124, 124 * t, 124 * t + 124) for t in range(8)] + \
        [(92, 124, 992, 1024)]
NCORES = 8
MULT = mybir.AluOpType.mult
ADD = mybir.AluOpType.add
MAXOP = mybir.AluOpType.max
ABS = mybir.ActivationFunctionType.Abs
COPY = mybir.ActivationFunctionType.Copy
IDENT = mybir.ActivationFunctionType.Identity


# --------------- custom DVE ops (registered once per process) -------------- #

def _register(name, spec):
    if name in _SUB_OPCODE_FOR_NAME:
        for op in OPS:
            if op.name == name:
                return op
    shas = {}
    for ver in ("v3", "v4"):
        s = DveOpSpec(name=name, opcode=0, uops=lower(spec, ver=ver), rd1_en=False)
        shas[ver] = s.sha(ver)
    op = DveOp(name, spec, subdim=False, uops_sha=shas)
    OPS.append(op)
    CUSTOM_DVE_SPECS[name] = spec
    _SUB_OPCODE_FOR_NAME[name] = _CUSTOM_DVE_ROW_BASE + len(OPS) - 1
    return op


# (in0 + in1) * imm2; rounding/clipping via u8 writeback (saturating RNE)
ADD_SCALE_ANT = _register("ADD_SCALE_ANT", Spec(
    body=(Src0 + Src1) * C2,
    reference=lambda in0, in1, s0, s1, imm2: (in0 + in1) * imm2,
))

# |in0|; round/clip via u8 writeback (the ISA rejects abs_max tensor_scalar)
ABS_ANT = _register("ABS_ANT", Spec(
    body=select(Src0 > C0, Src0, C0 - Src0),
    reference=lambda in0, in1, s0, s1, imm2: abs(in0),
))


# ------------------------------- bass program ------------------------------ #

def _band(c):
    """Shifted banded matrix: A[k, m] = c[k-m] for k-m in {0,1,2}:
    out[m] = sum_t c[t] * x[m+t] (3-tap vertical conv centered at row m+1)."""
    A = np.zeros((P, P), np.float32)
    for m in range(P):
        for t in range(3):
            if m + t < P:
                A[m + t, m] = c[t]
    return A


def _emit_filter(nc, ps, spec, rhs, halo=None):
    """Accumulating banded MMs for the two 512-col chunks of a [P, 1024]
    psum tile, grouped by band (LDW reuse). spec: [(band_tile, dx)] taps.
    halo: optional [P, 2] psum for the 2 extra lp columns (gauss only)."""
    writes = [0, 0, 0]
    total = len(spec)
    for bd, dx in spec:
        for c in range(2):
            first = writes[c] == 0
            writes[c] += 1
            last = writes[c] == total
            nc.tensor.matmul(ps[:, 512 * c:512 * (c + 1)], bd[:],
                             rhs[:, 512 * c + dx:512 * c + dx + 512],
                             start=first, stop=last)
        if halo is not None:
            first = writes[2] == 0
            writes[2] += 1
            last = writes[2] == total
            nc.tensor.matmul(halo[:, 0:2], bd[:],
                             rhs[:, 1024 + dx:1026 + dx],
                             start=first, stop=last)


def _build():
    nc = bacc.Bacc("TRN2", num_devices=NCORES)
    X = nc.dram_tensor("x", [ROWS, WP], f32r, kind="ExternalInput")
    BSEL = nc.dram_tensor("bsel", [1, 2], f32, kind="ExternalInput")
    O8 = nc.dram_tensor("o8", [6, 1024, W], u8, kind="ExternalOutput")
    NORMS = nc.dram_tensor("onorms", [1, 2], f32, kind="ExternalOutput")

    G1m = nc.inline_tensor(_band([1, 2, 1]) / 16.0, name="G1m")
    G2m = nc.inline_tensor(_band([2, 4, 2]) / 16.0, name="G2m")
    SB1m = nc.inline_tensor(_band([-1, 0, 1]), name="SB1m")
    SB2m = nc.inline_tensor(_band([-2, 0, 2]), name="SB2m")
    BPm = nc.inline_tensor(_band([1, 2, 1]), name="BPm")
    BNm = nc.inline_tensor(_band([-1, -2, -1]), name="BNm")
    DYm = nc.inline_tensor(_band([0, -1, 1]), name="DYm")
    bfb = lambda c, nm: nc.inline_tensor(_band(c).astype(ml_dtypes.bfloat16),
                                         name=nm)
    DPb = bfb([1, 0, -1], "DPb")
    BXPb = bfb([1, 1, 1], "BXPb")
    BXNb = bfb([-1, -1, -1], "BXNb")
    Ib = bfb([0, 1, 0], "Ib")
    SHNb = bfb([-1, 0, 0], "SHNb")
    L2b = bfb([2, 0, 2], "L2b")
    M8b = bfb([0, -8, 0], "M8b")

    with TileContext(nc) as tc:
        with tc.tile_pool(name="keep", bufs=1) as keep, \
             tc.tile_pool(name="xwork", bufs=NB) as xwork, \
             tc.tile_pool(name="qwork", bufs=2) as qwork, \
             tc.tile_pool(name="u8", bufs=2) as u8p, \
             tc.tile_pool(name="psum", bufs=3, space="PSUM") as psum, \
             tc.tile_pool(name="pst", bufs=2, space="PSUM") as pst, \
             tc.tile_pool(name="dram", bufs=1, space="DRAM") as dram:

            def load_const(t, dt_):
                tl = keep.tile([P, P], dt_, tag=t.name)
                nc.sync.dma_start(out=tl[:], in_=t.ap()[:, :])
                return tl

            # x bands over three DMA lanes: SP (t0 split, t2, t4, t6, t8),
            # ACT (t1, t3 then free for lp copies), Pool (t5, t7).
            masters = {}
            for m in (G1m, G2m):
                tl = keep.tile([P, P], f32, tag=m.name)
                nc.scalar.dma_start(out=tl[:], in_=m.ap()[:, :])
                masters[m.name] = tl
            xts = []
            for t in range(NB):
                xt = xwork.tile([P, WP], f32r, tag="xt")
                r0 = TSTART[t]
                if t == 0:
                    nc.sync.dma_start(out=xt[:, 0:514], in_=X[r0:r0 + P, 0:514])
                    nc.sync.dma_start(out=xt[:, 514:WP],
                                      in_=X[r0:r0 + P, 514:WP])
                elif t in (2, 4, 6, 8):
                    nc.sync.dma_start(out=xt[:], in_=X[r0:r0 + P, :])
                elif t in (1, 3):
                    nc.scalar.dma_start(out=xt[:], in_=X[r0:r0 + P, :])
                else:
                    nc.gpsimd.dma_start(out=xt[:], in_=X[r0:r0 + P, :])
                xts.append(xt)
            rbands = {}
            for nm in ("G1m", "G2m"):
                rt = keep.tile([P, P], f32r, tag=nm + "r")
                nc.vector.tensor_copy(out=rt[:], in_=masters[nm][:])
                rbands[nm] = rt
            b127 = keep.tile([P, 1], f32, tag="b127")
            nc.vector.tensor_scalar(out=b127[:], in0=rbands["G1m"][:, 0:1],
                                    scalar1=0.0, scalar2=127.5, op0=MULT,
                                    op1=ADD)

            for m in (SB1m, SB2m, BPm, BNm, DYm):
                masters[m.name] = load_const(m, f32)
            DPh = load_const(DPb, bf16)
            BXPh = load_const(BXPb, bf16)
            BXNh = load_const(BXNb, bf16)
            Ih = load_const(Ib, bf16)
            SHNh = load_const(SHNb, bf16)
            L2h = load_const(L2b, bf16)
            M8h = load_const(M8b, bf16)
            bsel = keep.tile([1, 2], f32, tag="bsel")
            nc.scalar.dma_start(out=bsel[:], in_=BSEL[:, :])

            for nm in ("SB1m", "SB2m", "BPm", "BNm", "DYm"):
                rt = keep.tile([P, P], f32r, tag=nm + "r")
                nc.vector.tensor_copy(out=rt[:], in_=masters[nm][:])
                rbands[nm] = rt
            G1r, G2r = rbands["G1m"], rbands["G2m"]
            SB1r, SB2r = rbands["SB1m"], rbands["SB2m"]
            BPr, BNr, DYr = rbands["BPm"], rbands["BNm"], rbands["DYm"]

            lps = []
            macc = keep.tile([P, 1], f32, tag="macc")

            # ====== phase A1: per band gauss (with 2-col halo) + max ====== #
            for t in range(NB):
                xt = xts[t]
                lp = keep.tile([P, LPW], f32r, tag=f"lp{t}")
                ps = psum.tile([P, W], f32, tag="ps")
                ph = pst.tile([P, 2], f32, tag="ph")
                _emit_filter(nc, ps[:], [(G1r, 0), (G1r, 2), (G2r, 1)], xt,
                             halo=ph[:])
                nc.scalar.activation(lp[:, 0:W], ps[:], COPY)
                nc.scalar.activation(lp[:, W:LPW], ph[:], COPY)
                mt = qwork.tile([P, 1], f32, tag="mt")
                nc.vector.tensor_reduce(out=mt[:], in_=lp[:, 0:W],
                                        axis=mybir.AxisListType.X, op=MAXOP)
                if t == 0:
                    nc.vector.tensor_copy(out=macc[:], in_=mt[:])
                else:
                    nc.vector.tensor_tensor(out=macc[:], in0=macc[:],
                                            in1=mt[:], op=MAXOP)
                lps.append(lp)

            # ---- norm: partition reduce + AllGather (max done locally) --- #
            pm = keep.tile([P, 1], f32, tag="pm")
            nc.gpsimd.partition_all_reduce(pm[:], macc[:], P,
                                           bass_isa.ReduceOp.max)
            m2 = keep.tile([1, 2], f32, tag="m2")
            nc.gpsimd.tensor_scalar(out=m2[:], in0=bsel[:], scalar1=pm[0:1, 0:1],
                                    scalar2=None, op0=MULT)
            ib = dram.tile([1, 2], f32)
            ob = dram.tile([NCORES, 2], f32)
            nc.sync.dma_start(ib[:], m2[:])
            nc.gpsimd.collective_compute(
                "AllGather", mybir.AluOpType.bypass,
                replica_groups=[list(range(NCORES))],
                ins=[ib.opt()], outs=[ob.opt()])

            # ---- norm-independent shadow work while AllGather runs ---- #
            sxs, sys_ = [], []
            for t in range(NB):
                j0, j1, g0, g1 = STORE[t]
                xt = xts[t]
                dyq = u8p.tile([P, W], u8, tag="dyq")
                pdy = psum.tile([P, W], f32, tag="ps")
                _emit_filter(nc, pdy[:], [(DYr, 2)], xt)
                nc.scalar.activation(dyq[:, 0:512], pdy[:, 0:512], IDENT,
                                     scale=127.5, bias=b127[:])
                nc.vector.tensor_scalar(out=dyq[:, 512:W], in0=pdy[:, 512:W],
                                        scalar1=127.5, scalar2=127.5,
                                        op0=MULT, op1=ADD)
                nc.sync.dma_start(out=O8[0, g0:g1, :], in_=dyq[j0:j1])

                sx = keep.tile([P, W], f16, tag=f"sx{t}")
                sy = keep.tile([P, W], f16, tag=f"sy{t}")
                px = psum.tile([P, W], f32, tag="ps")
                _emit_filter(nc, px[:], [(SB1r, 0), (SB1r, 2), (SB2r, 1)],
                             lps[t])
                nc.scalar.activation(sx[:], px[:], ABS)
                py = psum.tile([P, W], f32, tag="ps")
                _emit_filter(nc, py[:], [(BNr, 0), (BPr, 2)], lps[t])
                nc.scalar.activation(sy[:], py[:], ABS)
                sxs.append(sx)
                sys_.append(sy)

            # ---- post-collective norm chain (Pool tiny ops, DVE rcp) ---- #
            norms_sb = keep.tile([NCORES, 2], f32, tag="norms_sb")
            nc.sync.dma_start(norms_sb[:], ob[:])
            nr = keep.tile([NCORES, 2], f32, tag="nr")
            nc.gpsimd.partition_all_reduce(nr[:], norms_sb[:], NCORES,
                                           bass_isa.ReduceOp.max)
            nc.sync.dma_start(NORMS[:, :], nr[0:1, :])
            nbv = keep.tile([1, 2], f32, tag="nbv")
            nc.gpsimd.tensor_tensor(out=nbv[:], in0=nr[0:1, :], in1=bsel[:],
                                    op=MULT)
            nb = keep.tile([1, 1], f32, tag="nb")
            nc.gpsimd.tensor_tensor(out=nb[:], in0=nbv[:, 0:1],
                                    in1=nbv[:, 1:2], op=ADD)
            rcp = keep.tile([1, 1], f32, tag="rcp")
            nc.vector.reciprocal(out=rcp[:], in_=nb[:])
            rb1 = keep.tile([1, 1], f32, tag="rb1")
            nc.vector.tensor_scalar(out=rb1[:], in0=rcp[:], scalar1=255.0,
                                    scalar2=None, op0=MULT)
            rb = keep.tile([P, 1], f32, tag="rb")
            nc.gpsimd.partition_broadcast(rb[:], rb1[:], P)

            # ========================= phase B ========================= #
            # software-pipelined: q/spf for band t+1 are emitted before band
            # t's epilogues so Pool's queue never delays the next band's spf
            qs, spfs = [], []

            def emit_qspf(t):
                q = qwork.tile([P, LPW], i32, tag="q")
                spf = qwork.tile([P, LPW], bf16, tag="spf")
                if t == 0:
                    # 2 column pieces on Pool+DVE so the first phase-B
                    # matmul starts one piece after rb lands
                    for i, (a, b) in enumerate(((0, 514), (514, LPW))):
                        eng = nc.gpsimd if i == 0 else nc.vector
                        eng.tensor_scalar(out=q[:, a:b], in0=lps[t][:, a:b],
                                          scalar1=rb[:], scalar2=-0.5,
                                          op0=MULT, op1=ADD)
                        eng.tensor_copy(out=spf[:, a:b], in_=q[:, a:b])
                else:
                    nc.gpsimd.tensor_scalar(out=q[:], in0=lps[t][:],
                                            scalar1=rb[:], scalar2=-0.5,
                                            op0=MULT, op1=ADD)
                    nc.gpsimd.tensor_copy(out=spf[:], in_=q[:])
                qs.append(q)
                spfs.append(spf)

            def emit_band(t):
                j0, j1, g0, g1 = STORE[t]
                xt = xts[t]
                spf = spfs[t]

                # sobel finalize: ax on ACT, ay on Pool, addw on DVE
                ax = u8p.tile([P, W], u8, tag="ax")
                nc.scalar.activation(ax[:], sxs[t][:], COPY, scale=rb[:])
                ay = u8p.tile([P, W], u8, tag="ay")
                nc.gpsimd.tensor_scalar(out=ay[:], in0=sys_[t][:],
                                        scalar1=rb[:], scalar2=None, op0=MULT)
                s8 = u8p.tile([P, W], u8, tag="s8")
                nc.vector._custom_dve(ADD_SCALE_ANT, out=s8[:],
                                      in0=ax[:], in1=ay[:], s0=0.0, s1=0.0,
                                      imm2=0.5)
                nc.sync.dma_start(out=O8[4, g0:g1, :], in_=s8[j0:j1])

                # dx: Pool sub -> f16, Pool affine -> u8
                dxf = u8p.tile([P, W], f16, tag="dxf")
                nc.gpsimd.tensor_tensor(out=dxf[:], in0=xt[:, 2:W + 2],
                                        in1=xt[:, 1:W + 1],
                                        op=mybir.AluOpType.subtract)
                dxq = u8p.tile([P, W], u8, tag="dxq")
                nc.gpsimd.tensor_scalar(out=dxq[:], in0=dxf[:], scalar1=127.5,
                                        scalar2=127.5, op0=MULT, op1=ADD)
                nc.sync.dma_start(out=O8[1, g0:g1, :], in_=dxq[j0 + 2:j1 + 2])

                # prewitt: px -> ACT Abs, py -> DVE custom |.|, addw on DVE
                pxq = u8p.tile([P, W], u8, tag="pxq")
                pyq = u8p.tile([P, W], u8, tag="pyq")
                pp = psum.tile([P, W], f32, tag="ps")
                _emit_filter(nc, pp[:], [(DPh, 0), (DPh, 1), (DPh, 2)], spf)
                nc.scalar.activation(pxq[:], pp[:], ABS)
                pp = psum.tile([P, W], f32, tag="ps")
                _emit_filter(nc, pp[:], [(BXNh, 0), (BXPh, 2)], spf)
                nc.vector._custom_dve(ABS_ANT, out=pyq[:], in0=pp[:],
                                      s0=0.0, s1=0.0, imm2=0.0)
                p8 = u8p.tile([P, W], u8, tag="p8")
                nc.vector._custom_dve(ADD_SCALE_ANT, out=p8[:],
                                      in0=pxq[:], in1=pyq[:], s0=0.0, s1=0.0,
                                      imm2=0.5)
                nc.sync.dma_start(out=O8[3, g0:g1, :], in_=p8[j0:j1])

                # roberts: rx -> ACT Abs, ry -> DVE custom |.|, addw on DVE
                rxq = u8p.tile([P, W], u8, tag="rxq")
                ryq = u8p.tile([P, W], u8, tag="ryq")
                pr_ = psum.tile([P, W], f32, tag="ps")
                _emit_filter(nc, pr_[:], [(SHNh, 0), (Ih, 1)], spf)
                nc.scalar.activation(rxq[:], pr_[:], ABS)
                pr_ = psum.tile([P, W], f32, tag="ps")
                _emit_filter(nc, pr_[:], [(Ih, 0), (SHNh, 1)], spf)
                nc.vector._custom_dve(ABS_ANT, out=ryq[:], in0=pr_[:],
                                      s0=0.0, s1=0.0, imm2=0.0)
                r8 = u8p.tile([P, W], u8, tag="r8")
                nc.vector._custom_dve(ADD_SCALE_ANT, out=r8[:],
                                      in0=rxq[:], in1=ryq[:], s0=0.0, s1=0.0,
                                      imm2=0.5)
                nc.sync.dma_start(out=O8[2, g0:g1, :], in_=r8[j0:j1])

                # laplace: -> ACT Abs -> u8
                al = u8p.tile([P, W], u8, tag="al")
                pl = psum.tile([P, W], f32, tag="ps")
                _emit_filter(nc, pl[:], [(L2h, 0), (L2h, 2), (M8h, 1)], spf)
                nc.scalar.activation(al[:], pl[:], ABS)
                nc.sync.dma_start(out=O8[5, g0:g1, :], in_=al[j0:j1])

            for t in range(NB):
                emit_qspf(t)
                emit_band(t)
    return nc


# ------------------------------ PJRT runner ------------------------------- #

_CACHE = {}


def _get_fn():
    if "fn" in _CACHE:
        return _CACHE["fn"]
    import jax
    from jax.sharding import Mesh, PartitionSpec
    from jax.experimental.shard_map import shard_map
    from concourse import bass2jax
    from concourse.bass2jax import _bass_exec_p, partition_id_tensor

    nc = _build()
    nc.compile()
    bass2jax.install_neuronx_cc_hook()
    partition_name = nc.partition_id_tensor.name if nc.partition_id_tensor else None
    in_names, out_names, out_avals, zero_outs = [], [], [], []
    for alloc in nc.m.functions[0].allocations:
        if not isinstance(alloc, mybir.MemoryLocationSet):
            continue
        name = alloc.memorylocations[0].name
        if alloc.kind == "ExternalInput":
            if name != partition_name:
                in_names.append(name)
        elif alloc.kind == "ExternalOutput":
            shape = tuple(alloc.tensor_shape)
            dtype = mybir.dt.np(alloc.dtype)
            out_names.append(name)
            out_avals.append(jax.core.ShapedArray(shape, dtype))
            zero_outs.append(np.zeros(shape, dtype))
    n_params = len(in_names)
    all_in_names = list(in_names) + list(out_names)
    if partition_name is not None:
        all_in_names.append(partition_name)

    def _body(*args):
        operands = list(args)
        if partition_name is not None:
            operands.append(partition_id_tensor())
        outs = _bass_exec_p.bind(
            *operands,
            out_avals=tuple(out_avals),
            in_names=tuple(all_in_names),
            out_names=tuple(out_names),
            lowering_input_output_aliases=(),
            sim_require_finite=False,
            sim_require_nnan=False,
            nc=nc,
        )
        return tuple(outs)

    devices = jax.devices()[:NCORES]
    mesh = Mesh(np.asarray(devices), ("core",))
    in_specs = (PartitionSpec("core"),) * (n_params + len(out_names))
    out_specs = (PartitionSpec("core"),) * len(out_names)
    fn = jax.jit(
        shard_map(_body, mesh=mesh, in_specs=in_specs, out_specs=out_specs,
                  check_rep=False),
        keep_unused=True,
        donate_argnums=tuple(range(n_params, n_params + len(out_names))))
    info = dict(fn=fn, in_names=in_names, out_names=out_names,
                out_avals=out_avals, zero_outs=zero_outs, nc=nc)
    _CACHE["fn"] = info
    return info


def _host_inputs(pan):
    in_maps = []
    for c in range(NCORES):
        b, rh, ch = c // 4, (c % 4) // 2, c % 2
        pad = np.pad(pan[b, 0], 2, mode="reflect")  # [2052, 2052]
        Xc = np.ascontiguousarray(
            pad[1024 * rh:1024 * rh + ROWS, 1024 * ch:1024 * ch + WP])
        bs = np.zeros((1, 2), np.float32)
        bs[0, b] = 1.0
        in_maps.append({"x": Xc, "bsel": bs})
    return in_maps


def kernel(pan: np.ndarray) -> np.ndarray:
    pan = np.asarray(pan, dtype=np.float32)
    assert pan.shape == (2, 1, 2048, 2048), pan.shape
    info = _get_fn()
    in_maps = _host_inputs(pan)
    arrs = []
    for name in info["in_names"]:
        arrs.append(np.concatenate([in_maps[c][name] for c in range(NCORES)],
                                   axis=0))
    zeros = [np.zeros((NCORES * z.shape[0], *z.shape[1:]), z.dtype)
             for z in info["zero_outs"]]
    outs = info["fn"](*arrs, *zeros)
    byname = {nm: np.asarray(a) for nm, a in zip(info["out_names"], outs)}
    o8arr = byname["o8"].reshape(NCORES, 6, 1024, W)
    norms = byname["onorms"].reshape(NCORES, 1, 2)[0, 0]
    scales = (norms / np.float32(255.0)).astype(np.float32)
    out = np.empty((2, 6, 2048, 2048), np.float32)
    inv = np.float32(1.0 / 127.5)
    for c in range(NCORES):
        b, rh, ch = c // 4, (c % 4) // 2, c % 2
        sr = slice(1024 * rh, 1024 * rh + 1024)
        sc = slice(1024 * ch, 1024 * ch + 1024)
        o8 = o8arr[c].astype(np.float32)
        out[b, 0, sr, sc] = o8[0] * inv - 1.0
        out[b, 1, sr, sc] = o8[1] * inv - 1.0
        out[b, 2:6, sr, sc] = o8[2:6] * scales[b]
    out[:, 0, 0, :] = 0.0   # diff_y top row (replicate pad -> 0)
    out[:, 1, :, 0] = 0.0   # diff_x left col
    return out
